# revision 30
# baseline (speedup 1.0000x reference)
"""Hamming-similarity (BSC associative memory) kernel for 8 TRN2 NeuronCores.

reference: logit[b, c] = #matching bits between query[b] and am[c]
         = D - sum_d q - sum_d a + 2 * (q . a)
With bipolar x' = 2x - 1 in {-1, +1}:  (q' . a') = 2*logit - D, so
         logit = 0.5 * (q' @ a'^T) + D/2
One GEMM on +-1 data (exact in fp8) + scale/bias epilogue.

Sharding: data-parallel over the batch (4096 -> 512 per core), AM replicated.
The host pre-bipolarizes, casts to fp8 e4m3 (exact for +-1), pads D
10000 -> 10240 (80 chunks of 128) and classes 100 -> 128, and pre-swizzles
both operands into the exact SBUF layout [128 partitions, chunk-major
columns] so every DMA is 128 fat contiguous runs. Each core runs a raw
(non-Tile) Bass program: q DMAs streamed on the sync HWDGE queue, AM + out
DMAs on the scalar HWDGE queue, 40 fp8-DoubleRow matmuls (256 contraction
rows each) accumulating into one PSUM bank, DVE scale/bias epilogue. Each
core computes logit^T [100, 512]; the host concatenates and transposes.
"""

import numpy as np
import ml_dtypes

import concourse.bass as bass
import concourse.mybir as mybir
from concourse.bass_utils import run_bass_kernel_spmd

N_CORES = 8
BATCH = 4096
DIM = 10000
C = 100
C_PAD = 128           # class dim padded for DoubleRow AP alignment
B = BATCH // N_CORES  # 512 per core
P = 128
KC = 80               # contraction chunks of 128
D_PAD = KC * P        # 10240
# d-chunks per qT DMA. Sized so the matmul stream tracks the DMA stream
# without ever idling the PE > 3.4us (HAM re-throttle window), with small
# final groups so the post-DMA matmul tail is short.
GROUPS = [16, 20, 20, 16, 6, 2]
NG = len(GROUPS)
G_OFF = [sum(GROUPS[:i]) for i in range(NG + 1)]  # chunk offsets

_DT = mybir.dt.float8e4
_NPDT = ml_dtypes.float8_e4m3

_CACHE: dict = {}


def _make_bass():
    """Construct Bass without the __init__ const-AP memsets and all-engine
    barrier. The barrier makes every sequencer wait ~3.3us for the GpSimd Q7
    to boot; this kernel uses neither GpSimd nor the const APs, and all its
    cross-engine ordering runs through its own (load-time-zeroed) sems."""
    orig_barrier = bass.Bass.all_engine_barrier
    orig_memset = bass.BassSharedVectorInterface.memset
    bass.Bass.all_engine_barrier = lambda self, **kw: None
    bass.BassSharedVectorInterface.memset = lambda self, ap, c: None
    try:
        return bass.Bass()
    finally:
        bass.Bass.all_engine_barrier = orig_barrier
        bass.BassSharedVectorInterface.memset = orig_memset


def _build():
    nc = _make_bass()

    qT = nc.declare_dram_parameter("qT", [P, KC * B], _DT, isOutput=False)
    amT = nc.declare_dram_parameter("amT", [P, KC * C_PAD], _DT, isOutput=False)
    # out is 128 partitions (not 100) so the out-DMA uses all 16 SDMA lanes:
    # partial-lane DMAs fire part of their sem increment at dispatch (not
    # data-gated), which would make the final wait unsound. Host slices [:100].
    out = nc.declare_dram_parameter("out", [C_PAD, B], mybir.dt.float32, isOutput=True)

    with (
        nc.sbuf_tensor("q_sb", [P, KC * B], _DT) as q_sb,
        nc.sbuf_tensor("am_sb", [P, KC * C_PAD], _DT) as am_sb,
        nc.psum_tensor("acc", [C_PAD, B], mybir.dt.float32) as acc,
        nc.sbuf_tensor("out_sb", [C_PAD, B], mybir.dt.float32) as out_sb,
        nc.semaphore("asem") as asem,
        nc.semaphore("q0sem") as q0sem,
        nc.semaphore("q1sem") as q1sem,
        nc.semaphore("q2sem") as q2sem,
        nc.semaphore("q3sem") as q3sem,
        nc.semaphore("q4sem") as q4sem,
        nc.semaphore("q5sem") as q5sem,
        nc.semaphore("msem") as msem,
        nc.semaphore("esem") as esem,
        nc.semaphore("osem") as osem,
        nc.Block() as block,
    ):
        qsems = [q0sem, q1sem, q2sem, q3sem, q4sem, q5sem]
        assert len(qsems) == NG
        # chunked 3D views: DMA slices are contiguous per partition anyway
        q_mm = q_sb.ap().rearrange("p (o b) -> p o b", b=B)         # [128,80,512]
        am_mm = am_sb.ap().rearrange("p (o c) -> p o c", c=C_PAD)   # [128,80,128]

        # One dedicated semaphore per DMA: 16 increments on it mean exactly
        # "all 16 SDMA lanes of THIS transfer committed their data". A shared
        # counter would be racy: fast lanes bank increments from later groups
        # while a straggler lane still owes data for an earlier one.
        # q groups alternate between the two HWDGE rings (sync / scalar) so
        # per-ring queue depth halves and triggers issue twice as fast.
        q_dram = qT.ap().rearrange("p (o b) -> p o b", b=B)

        def q_dma(eng, g):
            eng.dma_start(
                out=q_mm[:, G_OFF[g] : G_OFF[g + 1], :],
                in_=q_dram[:, G_OFF[g] : G_OFF[g + 1], :],
            ).then_inc(qsems[g], 16)

        @block.sync
        def _(sync):
            # am fully lands first on this ring
            sync.dma_start(out=am_sb.ap(), in_=amT.ap()).then_inc(asem, 16)
            for g in range(0, NG, 2):
                q_dma(sync, g)

        @block.scalar
        def _(act):
            for g in range(1, NG, 2):
                q_dma(act, g)
            # out: two column-halves so the first DMA's descriptor-gen
            # overlaps the epilogue of the second half. Both are
            # 128-partition (full-lane) so their sem accounting is data-gated.
            act.wait_ge(esem, 1)
            act.dma_start(
                out=out.ap()[:, : B // 2], in_=out_sb.ap()[:, : B // 2]
            ).then_inc(osem, 16)
            act.wait_ge(esem, 2)
            act.dma_start(
                out=out.ap()[:, B // 2 :], in_=out_sb.ap()[:, B // 2 :]
            ).then_inc(osem, 16)
            act.wait_ge(osem, 32)

        @block.tensor
        def _(pe):
            pe.wait_ge(asem, 16)
            for g in range(NG):
                pe.wait_ge(qsems[g], 16)
                for k in range(G_OFF[g], G_OFF[g + 1], 2):
                    mm = pe.matmul(
                        acc.ap(),
                        am_mm[:, k : k + 2, :],
                        q_mm[:, k : k + 2, :],
                        start=(k == 0),
                        stop=(k == KC - 2),
                        perf_mode=mybir.MatmulPerfMode.DoubleRow,
                    )
            mm.then_inc(msem)

        @block.vector
        def _(dve):
            dve.wait_ge(msem, 1)
            for half in range(2):
                cols = slice(half * (B // 2), (half + 1) * (B // 2))
                dve.tensor_scalar(
                    out_sb.ap()[:, cols],
                    acc.ap()[:, cols],
                    0.5,
                    float(DIM) / 2.0,
                    mybir.AluOpType.mult,
                    mybir.AluOpType.add,
                ).then_inc(esem)

    return nc


def _get_nc():
    if "nc" not in _CACHE:
        _CACHE["nc"] = _build()
    return _CACHE["nc"]


def _swizzle(matT: np.ndarray, cols: int) -> np.ndarray:
    """[rows<=D_PAD, cols] bipolar f32 -> fp8 [128, KC*cols] chunk-major."""
    full = np.zeros((D_PAD, cols), dtype=_NPDT)
    full[: matT.shape[0]] = matT.astype(_NPDT)
    # [KC, 128, cols] -> [128, KC, cols] -> [128, KC*cols]
    return np.ascontiguousarray(
        full.reshape(KC, P, cols).transpose(1, 0, 2).reshape(P, KC * cols)
    )


def _prep_inputs(query: np.ndarray, am: np.ndarray):
    query = np.asarray(query, dtype=np.float32)
    am = np.asarray(am, dtype=np.float32)

    am_pad = np.zeros((C_PAD, DIM), dtype=np.float32)
    am_pad[:C] = 2.0 * am - 1.0
    amT_s = _swizzle(am_pad.T, C_PAD)

    in_maps = []
    for i in range(N_CORES):
        q_i = query[i * B : (i + 1) * B]  # [512, 10000]
        qT_s = _swizzle((2.0 * q_i - 1.0).T, B)
        in_maps.append({"qT": qT_s, "amT": amT_s})
    return in_maps


def _run(query: np.ndarray, am: np.ndarray, **kwargs):
    in_maps = _prep_inputs(query, am)
    res = run_bass_kernel_spmd(_get_nc(), in_maps, list(range(N_CORES)), **kwargs)
    logitT = np.concatenate(
        [res.results[i]["out"][:C] for i in range(N_CORES)], axis=1
    )  # [100, 4096]
    return np.ascontiguousarray(logitT.T).astype(np.float32), res


def kernel(query: np.ndarray, am: np.ndarray) -> np.ndarray:
    out, _ = _run(query, am)
    return out


# revision 31
# speedup vs baseline: 1.1845x; 1.1845x over previous
"""Hamming-similarity (BSC associative memory) kernel for 8 TRN2 NeuronCores.

reference: logit[b, c] = #matching bits between query[b] and am[c]
         = D - sum_d q - sum_d a + 2 * (q . a)
With bipolar x' = 2x - 1 in {-1, +1}:  (q' . a') = 2*logit - D, so
         logit = 0.5 * (q' @ a'^T) + D/2
One GEMM on +-1 data (exact in fp8) + scale/bias epilogue.

Sharding: data-parallel over the batch (4096 -> 512 per core), AM replicated.
The host pre-bipolarizes, casts to fp8 e4m3 (exact for +-1), pads D
10000 -> 10240 (80 chunks of 128) and classes 100 -> 128, and pre-swizzles
both operands into the exact SBUF layout [128 partitions, chunk-major
columns] so every DMA is 128 fat contiguous runs. Each core runs a raw
(non-Tile) Bass program: q DMAs streamed on the sync HWDGE queue, AM + out
DMAs on the scalar HWDGE queue, 40 fp8-DoubleRow matmuls (256 contraction
rows each) accumulating into one PSUM bank, DVE scale/bias epilogue. Each
core computes logit^T [100, 512]; the host concatenates and transposes.
"""

import numpy as np
import ml_dtypes

import concourse.bass as bass
import concourse.mybir as mybir
from concourse.bass_utils import run_bass_kernel_spmd

N_CORES = 8
BATCH = 4096
DIM = 10000
C = 100
C_PAD = 128           # class dim padded for DoubleRow AP alignment
B = BATCH // N_CORES  # 512 per core
P = 128
KC = 80               # contraction chunks of 128
D_PAD = KC * P        # 10240
# d-chunks per qT DMA. Sized so the matmul stream tracks the DMA stream
# without ever idling the PE > 3.4us (HAM re-throttle window), with small
# final groups so the post-DMA matmul tail is short.
GROUPS = [16, 20, 20, 16, 6, 2]
NG = len(GROUPS)
G_OFF = [sum(GROUPS[:i]) for i in range(NG + 1)]  # chunk offsets

_DT = mybir.dt.float8e4
_NPDT = ml_dtypes.float8_e4m3

_CACHE: dict = {}


def _make_bass():
    """Construct Bass without the __init__ const-AP memsets and all-engine
    barrier. The barrier makes every sequencer wait ~3.3us for the GpSimd Q7
    to boot; this kernel uses neither GpSimd nor the const APs, and all its
    cross-engine ordering runs through its own (load-time-zeroed) sems."""
    orig_barrier = bass.Bass.all_engine_barrier
    orig_memset = bass.BassSharedVectorInterface.memset
    bass.Bass.all_engine_barrier = lambda self, **kw: None
    bass.BassSharedVectorInterface.memset = lambda self, ap, c: None
    try:
        return bass.Bass()
    finally:
        bass.Bass.all_engine_barrier = orig_barrier
        bass.BassSharedVectorInterface.memset = orig_memset


def _build():
    nc = _make_bass()

    qT = nc.declare_dram_parameter("qT", [P, KC * B], _DT, isOutput=False)
    amT = nc.declare_dram_parameter("amT", [P, KC * C_PAD], _DT, isOutput=False)
    # out is 128 partitions (not 100) so the out-DMA uses all 16 SDMA lanes:
    # partial-lane DMAs fire part of their sem increment at dispatch (not
    # data-gated), which would make the final wait unsound. Host slices [:100].
    out = nc.declare_dram_parameter("out", [C_PAD, B], mybir.dt.float32, isOutput=True)

    with (
        nc.sbuf_tensor("q_sb", [P, KC * B], _DT) as q_sb,
        nc.sbuf_tensor("am_sb", [P, KC * C_PAD], _DT) as am_sb,
        nc.psum_tensor("acc", [C_PAD, B], mybir.dt.float32) as acc,
        nc.sbuf_tensor("out_sb", [C_PAD, B], mybir.dt.float32) as out_sb,
        nc.semaphore("asem") as asem,
        nc.semaphore("q0sem") as q0sem,
        nc.semaphore("q1sem") as q1sem,
        nc.semaphore("q2sem") as q2sem,
        nc.semaphore("q3sem") as q3sem,
        nc.semaphore("q4sem") as q4sem,
        nc.semaphore("q5sem") as q5sem,
        nc.semaphore("msem") as msem,
        nc.semaphore("esem") as esem,
        nc.semaphore("osem") as osem,
        nc.Block() as block,
    ):
        qsems = [q0sem, q1sem, q2sem, q3sem, q4sem, q5sem]
        assert len(qsems) == NG
        # chunked 3D views: DMA slices are contiguous per partition anyway
        q_mm = q_sb.ap().rearrange("p (o b) -> p o b", b=B)         # [128,80,512]
        am_mm = am_sb.ap().rearrange("p (o c) -> p o c", c=C_PAD)   # [128,80,128]

        # One dedicated semaphore per DMA: 16 increments on it mean exactly
        # "all 16 SDMA lanes of THIS transfer committed their data". A shared
        # counter would be racy: fast lanes bank increments from later groups
        # while a straggler lane still owes data for an earlier one.
        # All input DMAs ride ONE ring so groups complete strictly in order
        # at full bandwidth each (a second parallel ring just makes every
        # group finish later).
        q_dram = qT.ap().rearrange("p (o b) -> p o b", b=B)

        @block.sync
        def _(sync):
            # am fully lands first, then q groups in order
            sync.dma_start(out=am_sb.ap(), in_=amT.ap()).then_inc(asem, 16)
            for g in range(NG):
                sync.dma_start(
                    out=q_mm[:, G_OFF[g] : G_OFF[g + 1], :],
                    in_=q_dram[:, G_OFF[g] : G_OFF[g + 1], :],
                ).then_inc(qsems[g], 16)

        @block.scalar
        def _(act):
            # out on the idle scalar ring: two column-halves so the first
            # DMA's descriptor-gen overlaps the epilogue of the second half.
            # Both are 128-partition (full-lane) so their sem accounting is
            # data-gated.
            act.wait_ge(esem, 1)
            act.dma_start(
                out=out.ap()[:, : B // 2], in_=out_sb.ap()[:, : B // 2]
            ).then_inc(osem, 16)
            act.wait_ge(esem, 2)
            act.dma_start(
                out=out.ap()[:, B // 2 :], in_=out_sb.ap()[:, B // 2 :]
            ).then_inc(osem, 16)
            act.wait_ge(osem, 32)

        @block.tensor
        def _(pe):
            pe.wait_ge(asem, 16)
            for g in range(NG):
                pe.wait_ge(qsems[g], 16)
                for k in range(G_OFF[g], G_OFF[g + 1], 2):
                    mm = pe.matmul(
                        acc.ap(),
                        am_mm[:, k : k + 2, :],
                        q_mm[:, k : k + 2, :],
                        start=(k == 0),
                        stop=(k == KC - 2),
                        perf_mode=mybir.MatmulPerfMode.DoubleRow,
                    )
            mm.then_inc(msem)

        @block.vector
        def _(dve):
            dve.wait_ge(msem, 1)
            for half in range(2):
                cols = slice(half * (B // 2), (half + 1) * (B // 2))
                dve.tensor_scalar(
                    out_sb.ap()[:, cols],
                    acc.ap()[:, cols],
                    0.5,
                    float(DIM) / 2.0,
                    mybir.AluOpType.mult,
                    mybir.AluOpType.add,
                ).then_inc(esem)

    return nc


def _get_nc():
    if "nc" not in _CACHE:
        _CACHE["nc"] = _build()
    return _CACHE["nc"]


def _swizzle(matT: np.ndarray, cols: int) -> np.ndarray:
    """[rows<=D_PAD, cols] bipolar f32 -> fp8 [128, KC*cols] chunk-major."""
    full = np.zeros((D_PAD, cols), dtype=_NPDT)
    full[: matT.shape[0]] = matT.astype(_NPDT)
    # [KC, 128, cols] -> [128, KC, cols] -> [128, KC*cols]
    return np.ascontiguousarray(
        full.reshape(KC, P, cols).transpose(1, 0, 2).reshape(P, KC * cols)
    )


def _prep_inputs(query: np.ndarray, am: np.ndarray):
    query = np.asarray(query, dtype=np.float32)
    am = np.asarray(am, dtype=np.float32)

    am_pad = np.zeros((C_PAD, DIM), dtype=np.float32)
    am_pad[:C] = 2.0 * am - 1.0
    amT_s = _swizzle(am_pad.T, C_PAD)

    in_maps = []
    for i in range(N_CORES):
        q_i = query[i * B : (i + 1) * B]  # [512, 10000]
        qT_s = _swizzle((2.0 * q_i - 1.0).T, B)
        in_maps.append({"qT": qT_s, "amT": amT_s})
    return in_maps


def _run(query: np.ndarray, am: np.ndarray, **kwargs):
    in_maps = _prep_inputs(query, am)
    res = run_bass_kernel_spmd(_get_nc(), in_maps, list(range(N_CORES)), **kwargs)
    logitT = np.concatenate(
        [res.results[i]["out"][:C] for i in range(N_CORES)], axis=1
    )  # [100, 4096]
    return np.ascontiguousarray(logitT.T).astype(np.float32), res


def kernel(query: np.ndarray, am: np.ndarray) -> np.ndarray:
    out, _ = _run(query, am)
    return out


# revision 34
# speedup vs baseline: 1.3240x; 1.1178x over previous
"""Hamming-similarity (BSC associative memory) kernel for 8 TRN2 NeuronCores.

reference: logit[b, c] = #matching bits between query[b] and am[c]
         = D - sum_d q - sum_d a + 2 * (q . a)
With bipolar x' = 2x - 1 in {-1, +1}:  (q' . a') = 2*logit - D, so
         logit = 0.5 * (q' @ a'^T) + D/2
One GEMM on +-1 data (exact in fp8) + scale/bias epilogue.

Sharding: data-parallel over the batch (4096 -> 512 per core), AM replicated.
The host pre-bipolarizes, casts to fp8 e4m3 (exact for +-1), pads D
10000 -> 10240 (80 chunks of 128) and classes 100 -> 128, and pre-swizzles
both operands into the exact SBUF layout [128 partitions, chunk-major
columns] so every DMA is 128 fat contiguous runs. Each core runs a raw
(non-Tile) Bass program: q DMAs streamed on the sync HWDGE queue, AM + out
DMAs on the scalar HWDGE queue, 40 fp8-DoubleRow matmuls (256 contraction
rows each) accumulating into one PSUM bank, DVE scale/bias epilogue. Each
core computes logit^T [100, 512]; the host concatenates and transposes.
"""

import numpy as np
import ml_dtypes

import concourse.bass as bass
import concourse.mybir as mybir
from concourse.bass_utils import run_bass_kernel_spmd

N_CORES = 8
BATCH = 4096
DIM = 10000
C = 100
C_PAD = 128           # class dim padded for DoubleRow AP alignment
B = BATCH // N_CORES  # 512 per core
P = 128
KC = 80               # contraction chunks of 128
D_PAD = KC * P        # 10240
# d-chunks per qT DMA. Sized so the matmul stream tracks the DMA stream
# without ever idling the PE > 3.4us (HAM re-throttle window), with small
# final groups so the post-DMA matmul tail is short.
GROUPS = [16, 20, 20, 16, 6, 2]
NG = len(GROUPS)
G_OFF = [sum(GROUPS[:i]) for i in range(NG + 1)]  # chunk offsets

_DT = mybir.dt.float8e4
_NPDT = ml_dtypes.float8_e4m3

_CACHE: dict = {}


def _make_bass():
    """Construct Bass without the __init__ const-AP memsets and all-engine
    barrier. The barrier makes every sequencer wait ~3.3us for the GpSimd Q7
    to boot; this kernel uses neither GpSimd nor the const APs, and all its
    cross-engine ordering runs through its own (load-time-zeroed) sems."""
    orig_barrier = bass.Bass.all_engine_barrier
    orig_memset = bass.BassSharedVectorInterface.memset
    bass.Bass.all_engine_barrier = lambda self, **kw: None
    bass.BassSharedVectorInterface.memset = lambda self, ap, c: None
    try:
        return bass.Bass()
    finally:
        bass.Bass.all_engine_barrier = orig_barrier
        bass.BassSharedVectorInterface.memset = orig_memset


def _build():
    nc = _make_bass()

    qT = nc.declare_dram_parameter("qT", [P, KC * B], _DT, isOutput=False)
    amT = nc.declare_dram_parameter("amT", [P, KC * C_PAD], _DT, isOutput=False)
    # out is 128 partitions (not 100) so the out-DMA uses all 16 SDMA lanes:
    # partial-lane DMAs fire part of their sem increment at dispatch (not
    # data-gated), which would make the final wait unsound. Host slices [:100].
    out = nc.declare_dram_parameter("out", [C_PAD, B], mybir.dt.float32, isOutput=True)

    with (
        nc.sbuf_tensor("q_sb", [P, KC * B], _DT) as q_sb,
        nc.sbuf_tensor("am_sb", [P, KC * C_PAD], _DT) as am_sb,
        nc.sbuf_tensor("warm_w", [P, 2 * C_PAD], _DT) as warm_w,
        nc.sbuf_tensor("warm_x", [P, 2 * B], _DT) as warm_x,
        nc.psum_tensor("acc", [C_PAD, B], mybir.dt.float32) as acc,
        nc.psum_tensor("warm_ps", [C_PAD, B], mybir.dt.float32) as warm_ps,
        nc.sbuf_tensor("out_sb", [C_PAD, B], mybir.dt.float32) as out_sb,
        nc.semaphore("wsem") as wsem,
        nc.semaphore("asem") as asem,
        nc.semaphore("q0sem") as q0sem,
        nc.semaphore("q1sem") as q1sem,
        nc.semaphore("q2sem") as q2sem,
        nc.semaphore("q3sem") as q3sem,
        nc.semaphore("q4sem") as q4sem,
        nc.semaphore("q5sem") as q5sem,
        nc.semaphore("msem") as msem,
        nc.semaphore("esem") as esem,
        nc.semaphore("osem") as osem,
        nc.Block() as block,
    ):
        qsems = [q0sem, q1sem, q2sem, q3sem, q4sem, q5sem]
        assert len(qsems) == NG
        # chunked 3D views: DMA slices are contiguous per partition anyway
        q_mm = q_sb.ap().rearrange("p (o b) -> p o b", b=B)         # [128,80,512]
        am_mm = am_sb.ap().rearrange("p (o c) -> p o c", c=C_PAD)   # [128,80,128]

        # One dedicated semaphore per DMA: 16 increments on it mean exactly
        # "all 16 SDMA lanes of THIS transfer committed their data". A shared
        # counter would be racy: fast lanes bank increments from later groups
        # while a straggler lane still owes data for an earlier one.
        # All input DMAs ride ONE ring so groups complete strictly in order
        # at full bandwidth each (a second parallel ring just makes every
        # group finish later).
        q_dram = qT.ap().rearrange("p (o b) -> p o b", b=B)

        @block.sync
        def _(sync):
            # am fully lands first, then q groups in order
            sync.dma_start(out=am_sb.ap(), in_=amT.ap()).then_inc(asem, 16)
            for g in range(NG):
                sync.dma_start(
                    out=q_mm[:, G_OFF[g] : G_OFF[g + 1], :],
                    in_=q_dram[:, G_OFF[g] : G_OFF[g + 1], :],
                ).then_inc(qsems[g], 16)

        @block.scalar
        def _(act):
            # out on the idle scalar ring: two column-halves so the first
            # DMA's descriptor-gen overlaps the epilogue of the second half.
            # Both are 128-partition (full-lane) so their sem accounting is
            # data-gated.
            act.wait_ge(esem, 1)
            act.dma_start(
                out=out.ap()[:, : B // 2], in_=out_sb.ap()[:, : B // 2]
            ).then_inc(osem, 16)
            act.wait_ge(esem, 2)
            act.dma_start(
                out=out.ap()[:, B // 2 :], in_=out_sb.ap()[:, B // 2 :]
            ).then_inc(osem, 16)
            act.wait_ge(osem, 32)

        @block.tensor
        def _(pe):
            # HAM warm-up: ~18 throwaway matmuls on zeroed scratch while the
            # am/q0 DMAs stream in. Keeps the PE clock-gate open so the real
            # matmuls run warm from the first group (cold MMs are ~2x slower).
            pe.wait_ge(wsem, 1)
            ww = warm_w.ap().rearrange("p (o c) -> p o c", c=C_PAD)
            wx = warm_x.ap().rearrange("p (o b) -> p o b", b=B)
            for _ in range(18):
                pe.matmul(
                    warm_ps.ap(),
                    ww,
                    wx,
                    start=True,
                    stop=True,
                    perf_mode=mybir.MatmulPerfMode.DoubleRow,
                )
            pe.wait_ge(asem, 16)
            for g in range(NG):
                pe.wait_ge(qsems[g], 16)
                for k in range(G_OFF[g], G_OFF[g + 1], 2):
                    mm = pe.matmul(
                        acc.ap(),
                        am_mm[:, k : k + 2, :],
                        q_mm[:, k : k + 2, :],
                        start=(k == 0),
                        stop=(k == KC - 2),
                        perf_mode=mybir.MatmulPerfMode.DoubleRow,
                    )
            mm.then_inc(msem)

        @block.vector
        def _(dve):
            dve.memset(warm_w.ap(), 0)
            dve.memset(warm_x.ap(), 0).then_inc(wsem)
            dve.wait_ge(msem, 1)
            for half in range(2):
                cols = slice(half * (B // 2), (half + 1) * (B // 2))
                dve.tensor_scalar(
                    out_sb.ap()[:, cols],
                    acc.ap()[:, cols],
                    0.5,
                    float(DIM) / 2.0,
                    mybir.AluOpType.mult,
                    mybir.AluOpType.add,
                ).then_inc(esem)

    return nc


def _get_nc():
    if "nc" not in _CACHE:
        _CACHE["nc"] = _build()
    return _CACHE["nc"]


def _swizzle(matT: np.ndarray, cols: int) -> np.ndarray:
    """[rows<=D_PAD, cols] bipolar f32 -> fp8 [128, KC*cols] chunk-major."""
    full = np.zeros((D_PAD, cols), dtype=_NPDT)
    full[: matT.shape[0]] = matT.astype(_NPDT)
    # [KC, 128, cols] -> [128, KC, cols] -> [128, KC*cols]
    return np.ascontiguousarray(
        full.reshape(KC, P, cols).transpose(1, 0, 2).reshape(P, KC * cols)
    )


def _prep_inputs(query: np.ndarray, am: np.ndarray):
    query = np.asarray(query, dtype=np.float32)
    am = np.asarray(am, dtype=np.float32)

    am_pad = np.zeros((C_PAD, DIM), dtype=np.float32)
    am_pad[:C] = 2.0 * am - 1.0
    amT_s = _swizzle(am_pad.T, C_PAD)

    in_maps = []
    for i in range(N_CORES):
        q_i = query[i * B : (i + 1) * B]  # [512, 10000]
        qT_s = _swizzle((2.0 * q_i - 1.0).T, B)
        in_maps.append({"qT": qT_s, "amT": amT_s})
    return in_maps


def _run(query: np.ndarray, am: np.ndarray, **kwargs):
    in_maps = _prep_inputs(query, am)
    res = run_bass_kernel_spmd(_get_nc(), in_maps, list(range(N_CORES)), **kwargs)
    logitT = np.concatenate(
        [res.results[i]["out"][:C] for i in range(N_CORES)], axis=1
    )  # [100, 4096]
    return np.ascontiguousarray(logitT.T).astype(np.float32), res


def kernel(query: np.ndarray, am: np.ndarray) -> np.ndarray:
    out, _ = _run(query, am)
    return out
